# revision 58
# baseline (speedup 1.0000x reference)
import sys

sys.path.insert(0, "/opt/trn_rl_repo")
import numpy as np
import concourse.bass as bass
import concourse.tile as tile
from concourse import bacc, mybir
from concourse.bass_utils import run_bass_kernel_spmd

F32 = mybir.dt.float32
F32R = mybir.dt.float32r
I32 = mybir.dt.int32
AF = mybir.ActivationFunctionType
ALU = mybir.AluOpType

RECIP_MAGIC = 0x7EF127EA  # seed for 1/x bit trick; 1 Newton step after

B, T, C = 64, 500, 256
E, H, D = 512, 8, 64
N_CORES = 8
BL = B // N_CORES  # batches per core

USE_F32R = True  # matmul dtype switch (f32r: 4x faster, ~2e-4 matmul rel err)
USE_BF16 = True  # bf16 operands: FWL halves weight-load; ~0.5% quantization

TT = [128, 128, 128, 116]  # t/s tile sizes (500 = 3*128 + 116)

EXP_SPLIT = False  # two 512-col exp ACTs per pst tile vs one 1024-col


BF16 = mybir.dt.bfloat16
MM_DT = BF16 if USE_BF16 else (F32R if USE_F32R else F32)

_MODE = "full"  # amputation probe gate: full|noproj|noatt|noqkv


def _mm(ap):
    # reinterpret as the matmul dtype (no-op when already MM_DT / fp32 mode)
    if ap.dtype == MM_DT or USE_BF16:
        return ap
    return ap.bitcast(MM_DT)


def declare_io(nc):
    xw_dt = BF16 if USE_BF16 else F32
    xt = nc.dram_tensor("xt", [BL, C, T], xw_dt, kind="ExternalInput")
    wat = nc.dram_tensor("wat", [C, 3 * E], xw_dt, kind="ExternalInput")  # w_attn.T
    wpt = nc.dram_tensor("wpt", [E, E], xw_dt, kind="ExternalInput")  # w_proj.T
    bqk = nc.dram_tensor("bqk", [128, 8], F32, kind="ExternalInput")
    bvb = nc.dram_tensor("bvb", [128, E], F32, kind="ExternalInput")
    bpb = nc.dram_tensor("bpb", [128, E], F32, kind="ExternalInput")
    out = nc.dram_tensor("out", [BL, T, E], F32, kind="ExternalOutput")
    return xt, wat, wpt, bqk, bvb, bpb, out


def build_nc():
    nc = bacc.Bacc("TRN2", target_bir_lowering=False)
    tensors = declare_io(nc)
    with tile.TileContext(nc) as tc:
        _build_body(nc, tc, *tensors)
    nc.compile()
    return nc


def _build_body(nc, tc, xt, wat, wpt, bqk, bvb, bpb, out):
    from contextlib import ExitStack

    ctx = ExitStack()
    with ctx:
        cpool = ctx.enter_context(tc.tile_pool(name="consts", bufs=1))
        xpool = ctx.enter_context(tc.tile_pool(name="x", bufs=2))
        qkpool = ctx.enter_context(tc.tile_pool(name="qk", bufs=2))
        vpool = ctx.enter_context(tc.tile_pool(name="v", bufs=2))
        epool = ctx.enter_context(tc.tile_pool(name="est", bufs=3))
        ypool = ctx.enter_context(tc.tile_pool(name="yt", bufs=2))
        opool = ctx.enter_context(tc.tile_pool(name="os", bufs=2))
        zpool = ctx.enter_context(tc.tile_pool(name="zr", bufs=5))
        # PSUM pools: 8 banks total: io(pq/pv/po)[*,512]x2=2, st[*,1024]x2=4,
        # yt[65,500]x2=2
        ps_io = ctx.enter_context(tc.tile_pool(name="ps_io", bufs=2, space="PSUM"))
        ps_st = ctx.enter_context(tc.tile_pool(name="ps_st", bufs=2, space="PSUM"))
        ps_yt = ctx.enter_context(tc.tile_pool(name="ps_yt", bufs=2, space="PSUM"))

        # ---- constants ----
        wa = cpool.tile([128, 2 * 3 * E], MM_DT, name="wa")  # 2 c-ktiles x [128,1536]
        for k in range(2):
            nc.sync.dma_start(wa[:, k * 1536:(k + 1) * 1536], _mm(wat[k * 128:(k + 1) * 128, :]))
        bqk_t = cpool.tile([128, 8], F32, name="bqk_t")
        nc.sync.dma_start(bqk_t[:], bqk[:, :])
        # wp/bvb/bpb are DMA'd after load_x(0) (see prologue) so the first
        # batch's x isn't queued behind them
        wp = cpool.tile([128, 4 * E], MM_DT, name="wp")  # 4 e-ktiles x [128,512]
        bvb_t = cpool.tile([128, E], F32, name="bvb_t")
        bpb_t = cpool.tile([128, E], F32, name="bpb_t")

        def load_late_consts():
            for k in range(4):
                nc.sync.dma_start(wp[:, k * E:(k + 1) * E], _mm(wpt[k * 128:(k + 1) * 128, :]))
            nc.sync.dma_start(bvb_t[:], bvb[:, :])
            nc.sync.dma_start(bpb_t[:], bpb[:, :])
        rmagic = cpool.tile([1, T], I32, name="rmagic")
        nc.vector.memset(rmagic[:], RECIP_MAGIC)

        # pre-write the ones column (row 64 of each AV stationary block) into
        # both va pool buffers once; the per-batch v-bias add only touches
        # cols 0:64 of each 65-block so the ones persist across batches
        for i in range(2):
            va_init = vpool.tile([128, 4 * 520], MM_DT, name=f"va_init{i}", tag="va")
            va_v = va_init[:] if USE_BF16 else va_init[:].bitcast(F32)
            for s in range(4):
                v3 = va_v[:, s * 520:(s + 1) * 520].rearrange("p (h m) -> p h m", h=H)
                nc.vector.memset(v3[:, :, 64:65], 1.0)

        osb_cur = [None]

        def do_proj_mt(yt, b, mt):
            # out[t,f] = yT^T @ wpT + bproj, one 128-row t-chunk
            if _MODE == "noproj":
                return
            if mt == 0:
                osb_cur[0] = opool.tile([128, 4 * E], F32, name=f"osb{b}", tag="osb")
            osb = osb_cur[0]
            tt = TT[mt]
            po = ps_io.tile([128, E], F32, name=f"po{b}_{mt}", tag="ps_io")
            for k in range(4):
                nc.tensor.matmul(
                    po[0:tt, :],
                    _mm(yt[:, k * T + mt * 128:k * T + mt * 128 + tt]),
                    _mm(wp[:, k * E:(k + 1) * E]),
                    start=(k == 0), stop=(k == 3),
                )
            nc.vector.tensor_add(osb[0:tt, mt * E:(mt + 1) * E], po[0:tt, :], bpb_t[0:tt, :])
            nc.sync.dma_start(out[b, mt * 128:mt * 128 + tt, :], osb[0:tt, mt * E:(mt + 1) * E])

        def do_proj(yt, b):
            for mt in range(4):
                do_proj_mt(yt, b, mt)

        def load_x(b):
            # t-axis padded to stride 512 (pad cols zeroed) so downstream
            # matmuls can use full 512-col moving operands
            xtb = xpool.tile([128, 2 * 512], MM_DT, name=f"xtb{b}", tag="xtb")
            for k in range(2):
                nc.sync.dma_start(xtb[:, k * 512:k * 512 + T], _mm(xt[b, k * 128:(k + 1) * 128, :]))
                pad = xtb[:, k * 512 + T:(k + 1) * 512]
                if not USE_BF16 and pad.dtype != F32:
                    pad = pad.bitcast(F32)
                nc.vector.memset(pad, 0.0)
            return xtb

        SW = 1024  # per-s-tile col stride in the pair-packed est (2 x 512)
        PQO = (0, 4, 1, 5, 2, 6, 3, 7)

        xtbs = {}
        qks = {}
        vas = {}
        yts = {}

        def do_pq(b, m):
            if _MODE == "noqkv":
                return
            xtb = xtbs[b]
            pq = ps_io.tile([128, 512], F32, name=f"pq{b}_{m}", tag="ps_io")
            for k in range(2):
                nc.tensor.matmul(
                    pq[:],
                    _mm(wa[:, k * 1536 + m * 128:k * 1536 + (m + 1) * 128]),
                    _mm(xtb[:, k * 512:(k + 1) * 512]),
                    start=(k == 0), stop=(k == 1),
                )
            # add per-partition bias (b_attn for q/k) while copying to SBUF;
            # q-halves drain on ACT, k-halves on DVE so each ST pair's two
            # operands drain on both engines concurrently
            dst = qks[b][:, m * 512:(m + 1) * 512]
            if m % 2 == 0:
                nc.scalar.activation(dst, pq[:], AF.Identity,
                                     bias=bqk_t[:, m:m + 1])
            else:
                nc.vector.tensor_scalar(
                    dst, pq[:], bqk_t[:, m:m + 1], None, ALU.add)

        def do_pv(b, mt):
            if _MODE == "noqkv":
                return
            xtb = xtbs[b]
            tt = TT[mt]
            pv = ps_io.tile([128, E], F32, name=f"pv{b}_{mt}", tag="ps_io")
            for k in range(2):
                nc.tensor.matmul(
                    pv[0:tt, :],
                    _mm(xtb[:, k * 512 + mt * 128:k * 512 + mt * 128 + tt]),
                    _mm(wa[:, k * 1536 + 1024:k * 1536 + 1536]),
                    start=(k == 0), stop=(k == 1),
                )
            va3 = vas[b][:, mt * 520:(mt + 1) * 520].rearrange("p (h m) -> p h m", h=H)
            nc.vector.tensor_add(
                va3[0:tt, :, 0:64],
                pv[0:tt, :].rearrange("p (h m) -> p h m", h=H),
                bvb_t[0:tt, :].rearrange("p (h m) -> p h m", h=H),
            )

        def qkv_group(b, g):
            # g 0..3: one pq pair; g 4,5: one pv pair
            if g < 4:
                do_pq(b, PQO[2 * g])
                do_pq(b, PQO[2 * g + 1])
            else:
                do_pv(b, 2 * (g - 4))
                do_pv(b, 2 * (g - 4) + 1)

        def attention_phase(b):
            """Emit attention(b) interleaved with qkv(b+1) and proj(b-1) so
            the PE has fill work while ACT drains the exp tiles."""
            est = {}
            pyt = {}
            zbs = {}
            qk = qks[b]
            va = vas[b]
            nxt = b + 1 < BL
            prv = b - 1 >= 0

            def do_st_half(h0, h1, srange):
                # ST[s,t] = k_h @ q_h^T; exp(ST/8) -> est (heads h0/h1 packed
                # side by side in the free dim so one ACT covers both)
                if _MODE == "noatt":
                    return
                hp = h0 // 2
                if srange[0] == 0:
                    e2 = epool.tile([128, 4 * SW], MM_DT, name=f"est{b}_{hp}", tag="est")
                    est[h0] = e2
                    est[h1] = e2
                else:
                    e2 = est[h0]
                for s in srange:
                    st = TT[s]
                    pst = ps_st.tile([128, SW], F32, name=f"pst{b}_{hp}_{s}", tag="ps_st")
                    for idx, h in enumerate((h0, h1)):
                        jq, oq = h // 2, (h % 2) * 64
                        jk, ok = 4 + h // 2, (h % 2) * 64
                        nc.tensor.matmul(
                            pst[0:st, idx * 512:(idx + 1) * 512],
                            _mm(qk[ok:ok + 64, jk * 512 + s * 128:jk * 512 + s * 128 + st]),
                            _mm(qk[oq:oq + 64, jq * 512:(jq + 1) * 512]),
                            start=True, stop=True,
                        )
                    if EXP_SPLIT:
                        for idx in range(2):
                            nc.scalar.activation(
                                e2[0:st, s * SW + idx * 512:s * SW + (idx + 1) * 512],
                                pst[0:st, idx * 512:(idx + 1) * 512],
                                AF.Exp, scale=0.125)
                    else:
                        nc.scalar.activation(e2[0:st, s * SW:(s + 1) * SW],
                                             pst[0:st, :], AF.Exp, scale=0.125)

            def do_yt(h):
                # yT[d,t] (+ z in row 64) = [v_h | 1]^T @ expST, then 1/z
                if _MODE == "noatt":
                    return
                pyt[h] = ps_yt.tile([65, T], F32, name=f"pyt{b}_{h}", tag="ps_yt")
                p = pyt[h]
                e = est[h]
                off = (h % 2) * 512
                for s in range(4):
                    st = TT[s]
                    nc.tensor.matmul(
                        p[:],
                        _mm(va[0:st, s * 520 + 65 * h:s * 520 + 65 * h + 65]),
                        _mm(e[0:st, s * SW + off:s * SW + off + T]),
                        start=(s == 0), stop=(s == 3),
                    )
                # -1/z via bit-trick seed + one Newton step (DVE reciprocal is
                # ~3.4us per [1,T]; this chain is ~1us).  zm = (z*r0-2)*r0 is
                # MINUS the reciprocal; the sign is repaired by negating
                # w_proj on the host.
                zrow = p[64:65, :]
                r0 = zpool.tile([1, T], I32, name=f"r0{b}_{h}", tag="r0")
                nc.vector.scalar_tensor_tensor(
                    r0[:], zrow.bitcast(I32), -1, rmagic[:], ALU.mult, ALU.add)
                r0f = r0[:].bitcast(F32)
                zt = zpool.tile([1, T], F32, name=f"zt{b}_{h}", tag="zt")
                nc.vector.tensor_mul(zt[:], zrow, r0f)
                zm = zpool.tile([1, T], F32, name=f"zm{b}_{h}", tag="zm")
                nc.vector.scalar_tensor_tensor(
                    zm[:], zt[:], 2.0, r0f, ALU.subtract, ALU.mult)
                zs = zpool.tile([64, T], F32, name=f"zbs{b}_{h}", tag="zbs")
                zbs[h] = zs
                nc.gpsimd.partition_broadcast(zs[:], zm[:])

            def do_norm(h):
                if _MODE == "noatt":
                    return
                j, o = h // 2, (h % 2) * 64
                nc.vector.tensor_mul(
                    yt[o:o + 64, j * T:(j + 1) * T], pyt[h][0:64, :], zbs[h][:]
                )

            yt = ypool.tile([128, 4 * T], MM_DT, name=f"yt{b}", tag="yt")
            yts[b] = yt
            if _MODE == "noatt":
                ytf = yt[:].bitcast(F32) if yt.dtype != F32 else yt[:]
                nc.vector.memset(ytf, 0.5)

            # AV for pair hp-2 runs in part hp so exp(hp-2) has ~2 parts of
            # slack; norm(h) is further deferred one head so the gpsimd
            # z-broadcast lands before the DVE reaches the norm in its queue
            do_st_half(0, 1, (0, 1))
            if nxt:
                qkv_group(b + 1, 0)
            do_st_half(0, 1, (2, 3))
            do_st_half(2, 3, (0, 1))
            if nxt:
                qkv_group(b + 1, 1)
            do_st_half(2, 3, (2, 3))
            if prv:
                do_proj_mt(yts[b - 1], b - 1, 0)
            for hp in range(2, 4):
                do_st_half(2 * hp, 2 * hp + 1, (0, 1))
                if nxt:
                    qkv_group(b + 1, hp)
                do_yt(2 * hp - 4)
                if hp >= 3:
                    do_norm(2 * hp - 5)
                do_st_half(2 * hp, 2 * hp + 1, (2, 3))
                if prv:
                    do_proj_mt(yts[b - 1], b - 1, hp - 1)
                do_yt(2 * hp - 3)
                do_norm(2 * hp - 4)
            if b + 2 < BL:
                xtbs[b + 2] = load_x(b + 2)
            if nxt:
                qkv_group(b + 1, 4)
            do_yt(4)
            do_norm(3)
            if prv:
                do_proj_mt(yts[b - 1], b - 1, 3)
            do_yt(5)
            do_norm(4)
            if nxt:
                qkv_group(b + 1, 5)
            do_yt(6)
            do_norm(5)
            do_yt(7)
            do_norm(6)
            do_norm(7)
            if b - 2 >= 0:
                del yts[b - 2]

        # prologue: qkv(0) stands alone
        xtbs[0] = load_x(0)
        load_late_consts()
        qks[0] = qkpool.tile([128, 8 * 512], MM_DT, name="qk0", tag="qk")
        vas[0] = vpool.tile([128, 4 * 520], MM_DT, name="va0", tag="va")
        for g in range(6):
            qkv_group(0, g)
        if BL > 1:
            xtbs[1] = load_x(1)

        for b in range(BL):
            if b + 1 < BL:
                qks[b + 1] = qkpool.tile([128, 8 * 512], MM_DT, name=f"qk{b + 1}", tag="qk")
                vas[b + 1] = vpool.tile([128, 4 * 520], MM_DT, name=f"va{b + 1}", tag="va")
            attention_phase(b)
        do_proj(yts[BL - 1], BL - 1)


_NC = None


def _get_nc():
    global _NC
    if _NC is None:
        _NC = build_nc()
    return _NC


def prep_inputs(x, w_attn, b_attn, w_proj, b_proj):
    x = np.asarray(x, np.float32)
    w_attn = np.asarray(w_attn, np.float32)
    b_attn = np.asarray(b_attn, np.float32)
    w_proj = np.asarray(w_proj, np.float32)
    b_proj = np.asarray(b_proj, np.float32)

    import ml_dtypes
    xw_np = ml_dtypes.bfloat16 if USE_BF16 else np.float32
    xt_all = np.ascontiguousarray(x.transpose(0, 2, 1).astype(xw_np))  # [B, C, T]
    wat = np.ascontiguousarray(w_attn.T.astype(xw_np))  # [C, 1536]
    # negated: the on-chip z-chain produces -1/z (see do_yt), so yt = -y and
    # out = (-y) @ (-w_proj.T) + b_proj comes out right
    wpt = np.ascontiguousarray((-w_proj.T).astype(xw_np))  # [E, E]
    bqk = np.ascontiguousarray(b_attn[:1024].reshape(8, 128).T)  # [128, 8]
    bvb = np.ascontiguousarray(np.tile(b_attn[1024:1536][None, :], (128, 1)))
    bpb = np.ascontiguousarray(np.tile(b_proj[None, :], (128, 1)))

    in_maps = []
    for c in range(N_CORES):
        in_maps.append({
            "xt": np.ascontiguousarray(xt_all[c * BL:(c + 1) * BL]),
            "wat": wat, "wpt": wpt, "bqk": bqk, "bvb": bvb, "bpb": bpb,
        })
    return in_maps


def kernel(x, w_attn, b_attn, w_proj, b_proj):
    nc = _get_nc()
    in_maps = prep_inputs(x, w_attn, b_attn, w_proj, b_proj)
    res = run_bass_kernel_spmd(nc, in_maps, core_ids=list(range(N_CORES)))
    out = np.concatenate([res.results[c]["out"] for c in range(N_CORES)], axis=0)
    return out.astype(np.float32)



# revision 59
# speedup vs baseline: 1.0022x; 1.0022x over previous
import sys

sys.path.insert(0, "/opt/trn_rl_repo")
import numpy as np
import concourse.bass as bass
import concourse.tile as tile
from concourse import bacc, mybir
from concourse.bass_utils import run_bass_kernel_spmd

F32 = mybir.dt.float32
F32R = mybir.dt.float32r
I32 = mybir.dt.int32
AF = mybir.ActivationFunctionType
ALU = mybir.AluOpType

RECIP_MAGIC = 0x7EF127EA  # seed for 1/x bit trick; 1 Newton step after

B, T, C = 64, 500, 256
E, H, D = 512, 8, 64
N_CORES = 8
BL = B // N_CORES  # batches per core

USE_F32R = True  # matmul dtype switch (f32r: 4x faster, ~2e-4 matmul rel err)
USE_BF16 = True  # bf16 operands: FWL halves weight-load; ~0.5% quantization

TT = [128, 128, 128, 116]  # t/s tile sizes (500 = 3*128 + 116)

EXP_SPLIT = False  # two 512-col exp ACTs per pst tile vs one 1024-col


BF16 = mybir.dt.bfloat16
MM_DT = BF16 if USE_BF16 else (F32R if USE_F32R else F32)

_MODE = "full"  # amputation probe gate: full|noproj|noatt|noqkv


def _mm(ap):
    # reinterpret as the matmul dtype (no-op when already MM_DT / fp32 mode)
    if ap.dtype == MM_DT or USE_BF16:
        return ap
    return ap.bitcast(MM_DT)


def declare_io(nc):
    xw_dt = BF16 if USE_BF16 else F32
    xt = nc.dram_tensor("xt", [BL, C, T], xw_dt, kind="ExternalInput")
    wat = nc.dram_tensor("wat", [C, 3 * E], xw_dt, kind="ExternalInput")  # w_attn.T
    wpt = nc.dram_tensor("wpt", [E, E], xw_dt, kind="ExternalInput")  # w_proj.T
    bqk = nc.dram_tensor("bqk", [128, 8], F32, kind="ExternalInput")
    bvb = nc.dram_tensor("bvb", [128, E], F32, kind="ExternalInput")
    bpb = nc.dram_tensor("bpb", [128, E], F32, kind="ExternalInput")
    out = nc.dram_tensor("out", [BL, T, E], F32, kind="ExternalOutput")
    return xt, wat, wpt, bqk, bvb, bpb, out


def build_nc():
    nc = bacc.Bacc("TRN2", target_bir_lowering=False)
    tensors = declare_io(nc)
    with tile.TileContext(nc) as tc:
        _build_body(nc, tc, *tensors)
    nc.compile()
    return nc


def _build_body(nc, tc, xt, wat, wpt, bqk, bvb, bpb, out):
    from contextlib import ExitStack

    ctx = ExitStack()
    with ctx:
        cpool = ctx.enter_context(tc.tile_pool(name="consts", bufs=1))
        xpool = ctx.enter_context(tc.tile_pool(name="x", bufs=2))
        qkpool = ctx.enter_context(tc.tile_pool(name="qk", bufs=2))
        vpool = ctx.enter_context(tc.tile_pool(name="v", bufs=2))
        epool = ctx.enter_context(tc.tile_pool(name="est", bufs=3))
        ypool = ctx.enter_context(tc.tile_pool(name="yt", bufs=2))
        opool = ctx.enter_context(tc.tile_pool(name="os", bufs=2))
        zpool = ctx.enter_context(tc.tile_pool(name="zr", bufs=5))
        # PSUM pools: 8 banks total: io(pq/pv/po)[*,512]x2=2, st[*,1024]x2=4,
        # yt[65,500]x2=2
        ps_io = ctx.enter_context(tc.tile_pool(name="ps_io", bufs=2, space="PSUM"))
        ps_st = ctx.enter_context(tc.tile_pool(name="ps_st", bufs=2, space="PSUM"))
        ps_yt = ctx.enter_context(tc.tile_pool(name="ps_yt", bufs=2, space="PSUM"))

        # ---- constants ----
        wa = cpool.tile([128, 2 * 3 * E], MM_DT, name="wa")  # 2 c-ktiles x [128,1536]
        for k in range(2):
            nc.sync.dma_start(wa[:, k * 1536:(k + 1) * 1536], _mm(wat[k * 128:(k + 1) * 128, :]))
        bqk_t = cpool.tile([128, 8], F32, name="bqk_t")
        nc.sync.dma_start(bqk_t[:], bqk[:, :])
        # wp/bvb/bpb are DMA'd after load_x(0) (see prologue) so the first
        # batch's x isn't queued behind them
        wp = cpool.tile([128, 4 * E], MM_DT, name="wp")  # 4 e-ktiles x [128,512]
        bvb_t = cpool.tile([128, E], F32, name="bvb_t")
        bpb_t = cpool.tile([128, E], F32, name="bpb_t")

        def load_late_consts():
            for k in range(4):
                nc.sync.dma_start(wp[:, k * E:(k + 1) * E], _mm(wpt[k * 128:(k + 1) * 128, :]))
            nc.sync.dma_start(bvb_t[:], bvb[:, :])
            nc.sync.dma_start(bpb_t[:], bpb[:, :])
        rmagic = cpool.tile([1, T], I32, name="rmagic")
        nc.vector.memset(rmagic[:], RECIP_MAGIC)

        # pre-write the ones column (row 64 of each AV stationary block) into
        # both va pool buffers once; the per-batch v-bias add only touches
        # cols 0:64 of each 65-block so the ones persist across batches
        for i in range(2):
            va_init = vpool.tile([128, 4 * 520], MM_DT, name=f"va_init{i}", tag="va")
            va_v = va_init[:] if USE_BF16 else va_init[:].bitcast(F32)
            for s in range(4):
                v3 = va_v[:, s * 520:(s + 1) * 520].rearrange("p (h m) -> p h m", h=H)
                nc.vector.memset(v3[:, :, 64:65], 1.0)

        osb_cur = [None]

        def do_proj_mt(yt, b, mt):
            # out[t,f] = yT^T @ wpT + bproj, one 128-row t-chunk
            if _MODE == "noproj":
                return
            if mt == 0:
                osb_cur[0] = opool.tile([128, 4 * E], F32, name=f"osb{b}", tag="osb")
            osb = osb_cur[0]
            tt = TT[mt]
            po = ps_io.tile([128, E], F32, name=f"po{b}_{mt}", tag="ps_io")
            for k in range(4):
                nc.tensor.matmul(
                    po[0:tt, :],
                    _mm(yt[:, k * T + mt * 128:k * T + mt * 128 + tt]),
                    _mm(wp[:, k * E:(k + 1) * E]),
                    start=(k == 0), stop=(k == 3),
                )
            nc.vector.tensor_add(osb[0:tt, mt * E:(mt + 1) * E], po[0:tt, :], bpb_t[0:tt, :])
            nc.sync.dma_start(out[b, mt * 128:mt * 128 + tt, :], osb[0:tt, mt * E:(mt + 1) * E])

        def do_proj(yt, b):
            for mt in range(4):
                do_proj_mt(yt, b, mt)

        def load_x(b):
            # t-axis padded to stride 512 (pad cols zeroed) so downstream
            # matmuls can use full 512-col moving operands
            xtb = xpool.tile([128, 2 * 512], MM_DT, name=f"xtb{b}", tag="xtb")
            for k in range(2):
                nc.sync.dma_start(xtb[:, k * 512:k * 512 + T], _mm(xt[b, k * 128:(k + 1) * 128, :]))
                pad = xtb[:, k * 512 + T:(k + 1) * 512]
                if not USE_BF16 and pad.dtype != F32:
                    pad = pad.bitcast(F32)
                nc.vector.memset(pad, 0.0)
            return xtb

        SW = 1024  # per-s-tile col stride in the pair-packed est (2 x 512)
        PQO = (0, 4, 1, 5, 2, 6, 3, 7)

        xtbs = {}
        qks = {}
        vas = {}
        yts = {}

        def do_pq(b, m):
            if _MODE == "noqkv":
                return
            xtb = xtbs[b]
            pq = ps_io.tile([128, 512], F32, name=f"pq{b}_{m}", tag="ps_io")
            for k in range(2):
                nc.tensor.matmul(
                    pq[:],
                    _mm(wa[:, k * 1536 + m * 128:k * 1536 + (m + 1) * 128]),
                    _mm(xtb[:, k * 512:(k + 1) * 512]),
                    start=(k == 0), stop=(k == 1),
                )
            # add per-partition bias (b_attn for q/k) while copying to SBUF;
            # q-halves drain on ACT, k-halves on DVE so each ST pair's two
            # operands drain on both engines concurrently
            dst = qks[b][:, m * 512:(m + 1) * 512]
            if m % 2 == 0:
                nc.scalar.activation(dst, pq[:], AF.Identity,
                                     bias=bqk_t[:, m:m + 1])
            else:
                nc.vector.tensor_scalar(
                    dst, pq[:], bqk_t[:, m:m + 1], None, ALU.add)

        def do_pv(b, mt):
            if _MODE == "noqkv":
                return
            xtb = xtbs[b]
            tt = TT[mt]
            pv = ps_io.tile([128, E], F32, name=f"pv{b}_{mt}", tag="ps_io")
            for k in range(2):
                nc.tensor.matmul(
                    pv[0:tt, :],
                    _mm(xtb[:, k * 512 + mt * 128:k * 512 + mt * 128 + tt]),
                    _mm(wa[:, k * 1536 + 1024:k * 1536 + 1536]),
                    start=(k == 0), stop=(k == 1),
                )
            va3 = vas[b][:, mt * 520:(mt + 1) * 520].rearrange("p (h m) -> p h m", h=H)
            nc.vector.tensor_add(
                va3[0:tt, :, 0:64],
                pv[0:tt, :].rearrange("p (h m) -> p h m", h=H),
                bvb_t[0:tt, :].rearrange("p (h m) -> p h m", h=H),
            )

        def qkv_group(b, g):
            # g 0..3: one pq pair; g 4,5: one pv pair
            if g < 4:
                do_pq(b, PQO[2 * g])
                do_pq(b, PQO[2 * g + 1])
            else:
                do_pv(b, 2 * (g - 4))
                do_pv(b, 2 * (g - 4) + 1)

        def attention_phase(b):
            """Emit attention(b) interleaved with qkv(b+1) and proj(b-1) so
            the PE has fill work while ACT drains the exp tiles."""
            est = {}
            pyt = {}
            zbs = {}
            qk = qks[b]
            va = vas[b]
            nxt = b + 1 < BL
            prv = b - 1 >= 0

            def do_st_half(h0, h1, srange):
                # ST[s,t] = k_h @ q_h^T; exp(ST/8) -> est (heads h0/h1 packed
                # side by side in the free dim so one ACT covers both)
                if _MODE == "noatt":
                    return
                hp = h0 // 2
                if srange[0] == 0:
                    e2 = epool.tile([128, 4 * SW], MM_DT, name=f"est{b}_{hp}", tag="est")
                    est[h0] = e2
                    est[h1] = e2
                else:
                    e2 = est[h0]
                for s in srange:
                    st = TT[s]
                    pst = ps_st.tile([128, SW], F32, name=f"pst{b}_{hp}_{s}", tag="ps_st")
                    for idx, h in enumerate((h0, h1)):
                        jq, oq = h // 2, (h % 2) * 64
                        jk, ok = 4 + h // 2, (h % 2) * 64
                        nc.tensor.matmul(
                            pst[0:st, idx * 512:(idx + 1) * 512],
                            _mm(qk[ok:ok + 64, jk * 512 + s * 128:jk * 512 + s * 128 + st]),
                            _mm(qk[oq:oq + 64, jq * 512:(jq + 1) * 512]),
                            start=True, stop=True,
                        )
                    if EXP_SPLIT:
                        for idx in range(2):
                            nc.scalar.activation(
                                e2[0:st, s * SW + idx * 512:s * SW + (idx + 1) * 512],
                                pst[0:st, idx * 512:(idx + 1) * 512],
                                AF.Exp, scale=0.125)
                    else:
                        nc.scalar.activation(e2[0:st, s * SW:(s + 1) * SW],
                                             pst[0:st, :], AF.Exp, scale=0.125)

            def do_yt(h):
                # yT[d,t] (+ z in row 64) = [v_h | 1]^T @ expST, then 1/z
                if _MODE == "noatt":
                    return
                pyt[h] = ps_yt.tile([65, T], F32, name=f"pyt{b}_{h}", tag="ps_yt")
                p = pyt[h]
                e = est[h]
                off = (h % 2) * 512
                for s in range(4):
                    st = TT[s]
                    nc.tensor.matmul(
                        p[:],
                        _mm(va[0:st, s * 520 + 65 * h:s * 520 + 65 * h + 65]),
                        _mm(e[0:st, s * SW + off:s * SW + off + T]),
                        start=(s == 0), stop=(s == 3),
                    )
                # -1/z via bit-trick seed + one Newton step (DVE reciprocal is
                # ~3.4us per [1,T]; this chain is ~1us).  zm = (z*r0-2)*r0 is
                # MINUS the reciprocal; the sign is repaired by negating
                # w_proj on the host.
                zrow = p[64:65, :]
                r0 = zpool.tile([1, T], I32, name=f"r0{b}_{h}", tag="r0")
                nc.vector.scalar_tensor_tensor(
                    r0[:], zrow.bitcast(I32), -1, rmagic[:], ALU.mult, ALU.add)
                r0f = r0[:].bitcast(F32)
                zt = zpool.tile([1, T], F32, name=f"zt{b}_{h}", tag="zt")
                nc.vector.tensor_mul(zt[:], zrow, r0f)
                zm = zpool.tile([1, T], F32, name=f"zm{b}_{h}", tag="zm")
                nc.vector.scalar_tensor_tensor(
                    zm[:], zt[:], 2.0, r0f, ALU.subtract, ALU.mult)
                zs = zpool.tile([64, T], F32, name=f"zbs{b}_{h}", tag="zbs")
                zbs[h] = zs
                nc.gpsimd.partition_broadcast(zs[:], zm[:])

            def do_norm(h):
                if _MODE == "noatt":
                    return
                j, o = h // 2, (h % 2) * 64
                nc.vector.tensor_mul(
                    yt[o:o + 64, j * T:(j + 1) * T], pyt[h][0:64, :], zbs[h][:]
                )

            yt = ypool.tile([128, 4 * T], MM_DT, name=f"yt{b}", tag="yt")
            yts[b] = yt
            if _MODE == "noatt":
                ytf = yt[:].bitcast(F32) if yt.dtype != F32 else yt[:]
                nc.vector.memset(ytf, 0.5)

            # AV for pair hp-2 runs in part hp so exp(hp-2) has ~2 parts of
            # slack; norm(h) is further deferred one head so the gpsimd
            # z-broadcast lands before the DVE reaches the norm in its queue
            do_st_half(0, 1, (0, 1))
            if nxt:
                qkv_group(b + 1, 0)
            do_st_half(0, 1, (2, 3))
            do_st_half(2, 3, (0, 1))
            if nxt:
                qkv_group(b + 1, 1)
            do_st_half(2, 3, (2, 3))
            if prv:
                do_proj_mt(yts[b - 1], b - 1, 0)
            for hp in range(2, 4):
                do_st_half(2 * hp, 2 * hp + 1, (0, 1))
                do_yt(2 * hp - 4)
                if hp >= 3:
                    do_norm(2 * hp - 5)
                if nxt:
                    qkv_group(b + 1, hp)
                do_st_half(2 * hp, 2 * hp + 1, (2, 3))
                do_yt(2 * hp - 3)
                do_norm(2 * hp - 4)
                if prv:
                    do_proj_mt(yts[b - 1], b - 1, hp - 1)
            if b + 2 < BL:
                xtbs[b + 2] = load_x(b + 2)
            do_yt(4)
            do_norm(3)
            if nxt:
                qkv_group(b + 1, 4)
            do_yt(5)
            do_norm(4)
            if prv:
                do_proj_mt(yts[b - 1], b - 1, 3)
            do_yt(6)
            do_norm(5)
            if nxt:
                qkv_group(b + 1, 5)
            do_yt(7)
            do_norm(6)
            do_norm(7)
            if b - 2 >= 0:
                del yts[b - 2]

        # prologue: qkv(0) stands alone
        xtbs[0] = load_x(0)
        load_late_consts()
        qks[0] = qkpool.tile([128, 8 * 512], MM_DT, name="qk0", tag="qk")
        vas[0] = vpool.tile([128, 4 * 520], MM_DT, name="va0", tag="va")
        for g in range(6):
            qkv_group(0, g)
        if BL > 1:
            xtbs[1] = load_x(1)

        for b in range(BL):
            if b + 1 < BL:
                qks[b + 1] = qkpool.tile([128, 8 * 512], MM_DT, name=f"qk{b + 1}", tag="qk")
                vas[b + 1] = vpool.tile([128, 4 * 520], MM_DT, name=f"va{b + 1}", tag="va")
            attention_phase(b)
        do_proj(yts[BL - 1], BL - 1)


_NC = None


def _get_nc():
    global _NC
    if _NC is None:
        _NC = build_nc()
    return _NC


def prep_inputs(x, w_attn, b_attn, w_proj, b_proj):
    x = np.asarray(x, np.float32)
    w_attn = np.asarray(w_attn, np.float32)
    b_attn = np.asarray(b_attn, np.float32)
    w_proj = np.asarray(w_proj, np.float32)
    b_proj = np.asarray(b_proj, np.float32)

    import ml_dtypes
    xw_np = ml_dtypes.bfloat16 if USE_BF16 else np.float32
    xt_all = np.ascontiguousarray(x.transpose(0, 2, 1).astype(xw_np))  # [B, C, T]
    wat = np.ascontiguousarray(w_attn.T.astype(xw_np))  # [C, 1536]
    # negated: the on-chip z-chain produces -1/z (see do_yt), so yt = -y and
    # out = (-y) @ (-w_proj.T) + b_proj comes out right
    wpt = np.ascontiguousarray((-w_proj.T).astype(xw_np))  # [E, E]
    bqk = np.ascontiguousarray(b_attn[:1024].reshape(8, 128).T)  # [128, 8]
    bvb = np.ascontiguousarray(np.tile(b_attn[1024:1536][None, :], (128, 1)))
    bpb = np.ascontiguousarray(np.tile(b_proj[None, :], (128, 1)))

    in_maps = []
    for c in range(N_CORES):
        in_maps.append({
            "xt": np.ascontiguousarray(xt_all[c * BL:(c + 1) * BL]),
            "wat": wat, "wpt": wpt, "bqk": bqk, "bvb": bvb, "bpb": bpb,
        })
    return in_maps


def kernel(x, w_attn, b_attn, w_proj, b_proj):
    nc = _get_nc()
    in_maps = prep_inputs(x, w_attn, b_attn, w_proj, b_proj)
    res = run_bass_kernel_spmd(nc, in_maps, core_ids=list(range(N_CORES)))
    out = np.concatenate([res.results[c]["out"] for c in range(N_CORES)], axis=0)
    return out.astype(np.float32)

